# revision 1
# baseline (speedup 1.0000x reference)
"""FM (factorization machine) forward kernel for Trainium2, 8-core data
parallel, built around SBUF-resident tables + GPSIMD ``ap_gather``.

Reference computation (per batch row b with field indices x[b, 0..3]):
    xo      = x + field_offsets                      # global rows into tables
    e_f     = v[xo_f]        (16-dim embedding)      # per-field lookup
    bias_f  = bias[xo_f]     (scalar)
    s       = sum_f e_f ;  q = sum_f e_f^2
    y       = sigmoid( sum_f bias_f + 0.5 * sum_k (s_k^2 - q_k) )

Design (5.5x over the SWDGE dma_gather baseline, which was bound by
~13-23ns/descriptor DMA-ring processing of 1536 random 256B HBM reads):

* Tables live TRANSPOSED and SBUF-RESIDENT as packed fp16 pairs, one u32
  word per (row, dim):  [ v[e,k] | v[e,k]^2 - bias[e]/8 ].  Loaded once
  per NEFF execution; iterations gather SBUF-locally with GPSIMD
  ``ap_gather`` on all 8 Q7 cores in parallel (no HBM round trip).
* 16-partition group g serves batch rows g*64..g*64+63; partition 16g+k
  holds embedding dim k.  Per iteration: gather A = user table (31360
  rows, just under the 32767 signed-int16 index limit), gather B = item
  (6807) + merged genre-year combos (18*94=1692) in one 8499-row table.
* Pair algebra folds the bias INTO the quadratic term:
      s_tot[k] = v_u[k] + v_i[k] + s_m[k]
      t_tot[k] = q_u[k] + q_i[k] + q_m[k] - (b_u+b_i+b_m)/8
      z        = 0.5 * sum_k (s_tot^2 - t_tot)  ==  fm_term + bias_term
  since 0.5 * 16 * (b/8) = b.  DVE does 2 adds + square + sub in fp16;
  PE reduces the 16 dims per group with a single matmul against a
  stationary 0.5-block matrix (the 0.5 folds the FM scale in); ACT
  applies sigmoid from PSUM and issues the store.
* Engine pipeline (steady state ~3.5us, Pool-bound by the two ap_gather
  ucode calls): SP prefetches idx DMAs 3 deep; gather buffers, dfin, PSUM
  banks and y are double-buffered with semaphore slot guards.
"""

import numpy as np

N_CORES = 8
BATCH = 4096
ROWS = BATCH // N_CORES          # 512 rows per core
P = 128
G = 8                            # 16-partition groups (Q7 cores)
RPG = ROWS // G                  # 64 rows per group
K = 16
VU, VI, VG, VY = 31360, 6807, 18, 94
VM = VG * VY                     # 1692 merged genre-year combos
NB = VI + VM                     # 8499 rows in the item+merged table
NIDX = 3                         # idx staging buffers (SP prefetch depth)

_CACHE = {}


def _build(repeat=1):
    """Single-core Bass program (same program SPMD on all cores).

    repeat > 1 unrolls the body for steady-state timing; buffers rotate
    with depth-2 (depth-NIDX for idx) semaphore slot guards.
    """
    from contextlib import ExitStack

    import concourse.bacc as bacc
    import concourse.mybir as mybir
    from concourse.library_config import ap_gather as ap_gather_lib

    nc = bacc.Bacc("TRN2", debug=False)
    f32 = mybir.dt.float32
    f16 = mybir.dt.float16
    i16 = mybir.dt.int16
    i32 = mybir.dt.int32

    idx_d = nc.dram_tensor("idx16", [P, 12], i16, kind="ExternalInput")
    taba_d = nc.dram_tensor("taba", [P, VU], i32, kind="ExternalInput")
    tabb_d = nc.dram_tensor("tabb", [P, NB], i32, kind="ExternalInput")
    half_d = nc.dram_tensor("half", [P, G], f16, kind="ExternalInput")
    out_d = nc.dram_tensor("out", [G, RPG], f32, kind="ExternalOutput")

    with ExitStack() as ctx:
        taba_sb = ctx.enter_context(nc.sbuf_tensor([P, VU], i32))
        tabb_sb = ctx.enter_context(nc.sbuf_tensor([P, NB], i32))
        half_sb = ctx.enter_context(nc.sbuf_tensor([P, G], f16))
        idx_sb = [
            ctx.enter_context(nc.sbuf_tensor(f"idxb{i}", [P, 12], i16))
            for i in range(NIDX)
        ]
        ga = [
            ctx.enter_context(nc.sbuf_tensor(f"ga{i}", [P, RPG], i32))
            for i in range(2)
        ]
        gb = [
            ctx.enter_context(nc.sbuf_tensor(f"gb{i}", [P, 2 * RPG], i32))
            for i in range(2)
        ]
        tmp16 = ctx.enter_context(nc.sbuf_tensor([P, 2 * RPG], f16))
        tot16 = ctx.enter_context(nc.sbuf_tensor([P, 2 * RPG], f16))
        dsq = ctx.enter_context(nc.sbuf_tensor([P, RPG], f16))
        dfin = [
            ctx.enter_context(nc.sbuf_tensor(f"dfin{i}", [P, RPG], f16))
            for i in range(2)
        ]
        y = [
            ctx.enter_context(nc.sbuf_tensor(f"y{i}", [G, RPG], f32))
            for i in range(2)
        ]
        psum = [
            ctx.enter_context(nc.psum_tensor(f"ps{i}", [G, RPG], f32))
            for i in range(2)
        ]
        dmat = ctx.enter_context(nc.semaphore("dmat"))
        dmai = ctx.enter_context(nc.semaphore("dmai"))
        dmao = ctx.enter_context(nc.semaphore("dmao"))
        gsem = ctx.enter_context(nc.semaphore("gsem"))
        dvs = ctx.enter_context(nc.semaphore("dvs"))
        dvf = ctx.enter_context(nc.semaphore("dvf"))
        pes = ctx.enter_context(nc.semaphore("pes"))
        acts = ctx.enter_context(nc.semaphore("acts"))
        block = ctx.enter_context(nc.Block())

        taba3 = taba_sb[:].rearrange("p (e d) -> p e d", e=VU, d=1)
        tabb3 = tabb_sb[:].rearrange("p (e d) -> p e d", e=NB, d=1)

        @block.sync
        def _(sync):
            sync.dma_start(out=taba_sb[:], in_=taba_d[:]).then_inc(dmat, 16)
            sync.dma_start(out=tabb_sb[:], in_=tabb_d[:]).then_inc(dmat, 16)
            sync.dma_start(out=half_sb[:], in_=half_d[:]).then_inc(dmat, 16)
            for r in range(repeat):
                if r >= NIDX:
                    # idx slot r-NIDX consumed once Pool finished iter r-NIDX
                    sync.wait_ge(gsem, 2 * (r - NIDX + 1))
                sync.dma_start(
                    out=idx_sb[r % NIDX][:], in_=idx_d[:]
                ).then_inc(dmai, 16)

        @block.gpsimd
        def _(gpsimd):
            gpsimd.load_library(ap_gather_lib)
            gpsimd.wait_ge(dmat, 32)  # both tables resident
            for r in range(repeat):
                gpsimd.wait_ge(dmai, 16 * (r + 1))
                if r >= 2:
                    gpsimd.wait_ge(dvs, r - 1)  # DVE consumed bufs of r-2
                j = r % 2
                ga3 = ga[j][:].rearrange("p (i d) -> p i d", i=RPG, d=1)
                gb3 = gb[j][:].rearrange("p (i d) -> p i d", i=2 * RPG, d=1)
                gpsimd.ap_gather(
                    out_ap=ga3,
                    in_ap=taba3,
                    idxs_ap=idx_sb[r % NIDX][:, 0:4],
                    channels=P,
                    num_elems=VU,
                    d=1,
                    num_idxs=RPG,
                ).then_inc(gsem, 1)
                gpsimd.ap_gather(
                    out_ap=gb3,
                    in_ap=tabb3,
                    idxs_ap=idx_sb[r % NIDX][:, 4:12],
                    channels=P,
                    num_elems=NB,
                    d=1,
                    num_idxs=2 * RPG,
                ).then_inc(gsem, 1)

        @block.vector
        def _(vector):
            tot3 = tot16[:].rearrange("p (i two) -> p i two", i=RPG, two=2)
            s_v = tot3[:, :, 0]
            t_v = tot3[:, :, 1]
            with nc.allow_low_precision("fp16 FM pairs, abs err ~1e-4"):
                for r in range(repeat):
                    j = r % 2
                    ga16 = ga[j][:].bitcast(mybir.dt.float16)
                    gb16 = gb[j][:].bitcast(mybir.dt.float16)
                    vector.wait_ge(gsem, 2 * r + 2)  # both gathers landed
                    nc.vector.tensor_add(
                        tmp16[:],
                        gb16[:, 0:2 * RPG],
                        gb16[:, 2 * RPG:4 * RPG],
                    )
                    nc.vector.tensor_add(
                        tot16[:], tmp16[:], ga16
                    ).then_inc(dvs, 1)
                    nc.vector.tensor_mul(dsq[:], s_v, s_v)
                    if r >= 2:
                        vector.wait_ge(pes, r - 1)  # dfin r-2 consumed
                    nc.vector.tensor_sub(
                        dfin[j][:], dsq[:], t_v
                    ).then_inc(dvf, 1)

        @block.tensor
        def _(tensor):
            tensor.wait_ge(dmat, 48)  # half-block weights resident
            for r in range(repeat):
                j = r % 2
                tensor.wait_ge(dvf, r + 1)
                if r >= 2:
                    tensor.wait_ge(acts, r - 1)  # psum r-2 consumed
                nc.tensor.matmul(
                    psum[j][:], half_sb[:], dfin[j][:],
                    start=True, stop=True,
                ).then_inc(pes, 1)

        @block.scalar
        def _(scalar):
            import concourse.mybir as mybir_

            for r in range(repeat):
                j = r % 2
                scalar.wait_ge(pes, r + 1)
                if r >= 2:
                    scalar.wait_ge(dmao, 16 * (r - 1))  # store r-2 landed
                nc.scalar.activation(
                    out=y[j][:],
                    in_=psum[j][:],
                    func=mybir_.ActivationFunctionType.Sigmoid,
                ).then_inc(acts, 1)
                scalar.dma_start(out=out_d[:], in_=y[j][:]).then_inc(dmao, 16)
            scalar.wait_ge(dmao, 16 * repeat)

    nc.compile()
    return nc


def _prep_tables(v, bias):
    """Packed fp16-pair tables + 0.5-block PE weights (cached on v/bias)."""
    key = (id(v), id(bias))
    hit = _CACHE.get("tables")
    if hit is not None and hit[0] == key:
        return hit[1]
    v = np.asarray(v, dtype=np.float32)
    b = np.asarray(bias, dtype=np.float32)[:, 0]

    def pack(vrows, trows):
        # [n, 16] value + [n, 16] t -> [128, n] i32 (u32 word = [v16|t16])
        n = vrows.shape[0]
        f = np.empty((n, K, 2), np.float16)
        f[:, :, 0] = vrows.astype(np.float16)
        f[:, :, 1] = trows.astype(np.float16)
        u = f.view(np.uint32)[:, :, 0]          # [n, 16]
        return np.ascontiguousarray(
            np.tile(u.T, (G, 1)).view(np.int32)
        )                                        # [128, n]

    vu, vi = v[0:VU], v[VU:VU + VI]
    bu, bi = b[0:VU], b[VU:VU + VI]
    vg, vy = v[VU + VI:VU + VI + VG], v[VU + VI + VG:]
    bg, by = b[VU + VI:VU + VI + VG], b[VU + VI + VG:]

    def tq(vr, br):
        # consistent squares: square the fp16-rounded value
        v16 = vr.astype(np.float16).astype(np.float32)
        return v16 * v16 - br[:, None] / 8.0

    tab_a = pack(vu, tq(vu, bu))

    s_m = (vg[:, None, :] + vy[None, :, :]).reshape(VM, K)
    vg16 = vg.astype(np.float16).astype(np.float32)
    vy16 = vy.astype(np.float16).astype(np.float32)
    q_m = (vg16[:, None, :] ** 2 + vy16[None, :, :] ** 2).reshape(VM, K)
    b_m = (bg[:, None] + by[None, :]).reshape(VM)
    rows_b_v = np.concatenate([vi, s_m], axis=0)
    rows_b_t = np.concatenate(
        [tq(vi, bi), q_m - b_m[:, None] / 8.0], axis=0
    )
    tab_b = pack(rows_b_v, rows_b_t)

    half = np.zeros((P, G), np.float16)
    for g in range(G):
        half[16 * g:16 * (g + 1), g] = 0.5

    out = (tab_a, tab_b, half)
    _CACHE["tables"] = (key, out)
    return out


def _wrap_ag(codes):
    """[512] int -> [128, RPG//16] int16: idxs[16g+j, s] = codes[g*64+s*16+j]."""
    c = codes.astype(np.int16).reshape(G, RPG // 16, 16)   # [g, s, j]
    return np.ascontiguousarray(c.transpose(0, 2, 1).reshape(P, RPG // 16))


def _prep_inputs(x, v, bias):
    """Full inputs -> per-core in_maps."""
    x = np.asarray(x)
    tab_a, tab_b, half = _prep_tables(v, bias)
    in_maps = []
    for c in range(N_CORES):
        xc = x[c * ROWS:(c + 1) * ROWS].astype(np.int64)     # (512, 4)
        iu = _wrap_ag(xc[:, 0])                              # user codes
        ii = _wrap_ag(xc[:, 1])                              # item codes
        im = _wrap_ag(xc[:, 2] * VY + xc[:, 3] + VI)         # merged codes
        idx16 = np.concatenate([iu, ii, im], axis=1)         # [128, 12]
        in_maps.append(
            {"idx16": idx16, "taba": tab_a, "tabb": tab_b, "half": half}
        )
    return in_maps


def _assemble(results):
    """Per-core out[g, i] -> full (BATCH, 1) f32 output (row g*64+i)."""
    ys = []
    for c in range(N_CORES):
        o = np.asarray(results[c]["out"])                # (8, 64)
        ys.append(o.reshape(ROWS, 1))
    return np.concatenate(ys, axis=0).astype(np.float32)


def _get_exec(repeat=1):
    """Compile the SPMD program once; returns a cached jitted callable.

    Mirrors the multi-core branch of concourse.bass2jax.run_bass_via_pjrt
    but keeps the jitted function alive so repeat calls skip recompilation.
    """
    key = ("exec", repeat)
    if key in _CACHE:
        return _CACHE[key]
    import jax
    from jax.experimental.shard_map import shard_map
    from jax.sharding import Mesh, PartitionSpec

    import concourse.mybir as mybir
    from concourse import bass2jax

    bass2jax.install_neuronx_cc_hook()
    nc = _build(repeat)
    assert nc.dbg_addr is None
    partition_name = nc.partition_id_tensor.name if nc.partition_id_tensor else None

    in_names, out_names, out_avals = [], [], []
    for alloc in nc.m.functions[0].allocations:
        if not isinstance(alloc, mybir.MemoryLocationSet):
            continue
        name = alloc.memorylocations[0].name
        if alloc.kind == "ExternalInput":
            if name != partition_name:
                in_names.append(name)
        elif alloc.kind == "ExternalOutput":
            out_names.append(name)
            out_avals.append(
                jax.core.ShapedArray(
                    tuple(alloc.tensor_shape), mybir.dt.np(alloc.dtype)
                )
            )
    n_params, n_outs = len(in_names), len(out_names)
    all_names = in_names + out_names + ([partition_name] if partition_name else [])

    def _body(*args):
        operands = list(args)
        if partition_name is not None:
            operands.append(bass2jax.partition_id_tensor())
        outs = bass2jax._bass_exec_p.bind(
            *operands,
            out_avals=tuple(out_avals),
            in_names=tuple(all_names),
            out_names=tuple(out_names),
            lowering_input_output_aliases=(),
            sim_require_finite=True,
            sim_require_nnan=True,
            nc=nc,
        )
        return tuple(outs)

    devices = jax.devices()[:N_CORES]
    mesh = Mesh(np.asarray(devices), ("core",))
    fn = jax.jit(
        shard_map(
            _body,
            mesh=mesh,
            in_specs=(PartitionSpec("core"),) * (n_params + n_outs),
            out_specs=(PartitionSpec("core"),) * n_outs,
            check_rep=False,
        ),
        donate_argnums=tuple(range(n_params, n_params + n_outs)),
        keep_unused=True,
    )
    _CACHE[key] = (fn, in_names, out_names, out_avals, mesh)
    return _CACHE[key]


def _concat_inputs(x, v, bias, in_names):
    in_maps = _prep_inputs(x, v, bias)
    return [
        np.concatenate([in_maps[c][nm] for c in range(N_CORES)], axis=0)
        for nm in in_names
    ]


def _zero_outs(out_avals):
    return [
        np.zeros((N_CORES * av.shape[0], *av.shape[1:]), av.dtype)
        for av in out_avals
    ]


def run(x, v, bias, trace=False):
    """Returns (y, exec_time_ns_or_None)."""
    fn, in_names, out_names, out_avals, _ = _get_exec()
    outs = fn(*_concat_inputs(x, v, bias, in_names), *_zero_outs(out_avals))
    o = np.asarray(outs[out_names.index("out")]).reshape(N_CORES, G, RPG)
    return _assemble([{"out": o[c]} for c in range(N_CORES)]), None


def bench(x, v, bias, rounds=8, per_round=6, r1=8, r2=512):
    """Per-iteration kernel time via interleaved two-point unroll diff
    (cancels per-call RPC/dispatch overhead and slow drift)."""
    import time

    import jax
    from jax.sharding import NamedSharding, PartitionSpec

    def caller(repeat):
        fn, in_names, out_names, out_avals, mesh = _get_exec(repeat)
        sh = NamedSharding(mesh, PartitionSpec("core"))
        dev_in = [
            jax.device_put(a, sh)
            for a in _concat_inputs(x, v, bias, in_names)
        ]
        zeros = _zero_outs(out_avals)

        def call():
            return fn(*dev_in, *[jax.device_put(zz, sh) for zz in zeros])

        return call

    callA, callB = caller(r1), caller(r2)
    callA()[0].block_until_ready()
    callB()[0].block_until_ready()
    diffs = []
    for _ in range(rounds):
        tA, tB = [], []
        for _ in range(per_round):
            t0 = time.perf_counter()
            callA()[0].block_until_ready()
            tA.append(time.perf_counter() - t0)
        for _ in range(per_round):
            t0 = time.perf_counter()
            callB()[0].block_until_ready()
            tB.append(time.perf_counter() - t0)
        diffs.append(min(tB) - min(tA))
    diffs.sort()
    return diffs[len(diffs) // 2] / (r2 - r1) * 1e9


def kernel(x, v, bias):
    y, _ = run(x, v, bias, trace=False)
    return y

